# revision 1
# baseline (speedup 1.0000x reference)
"""Trainium2 Bass kernel for nn_InvariantHeadviaTP.

Reference computation (after dead-code elimination -- y1/y2/gates are never
used by the output):
    x0   = node_vec[:, :128]                  # [N, 128]
    a    = node_embedding                     # [N, 16]
    s0   = einsum('ni,na,iak->nk', x0, a, W1_l0[:, :, :128]) / sqrt(2048) + b1[:128]
    scal = silu(s0)                           # [N, 128]
    mid  = einsum('ni,na,iak->nk', scal, a, W2) / sqrt(2048) + b2   # [N, 16]
    h    = silu(mid @ W3 / 4 + b3)            # [N, 16]
    out  = h @ W4 / 4 + b4                    # [N, 1]

Strategy: data-parallel over 8 cores (2048 nodes each). Per core, work in a
transposed layout: features on SBUF partitions, nodes on the free dim.
The bilinear contractions over c=(a,i) [16*128=2048] are done as 16
PSUM-accumulated matmuls with bf16 operands:
    s0T[k, n] = sum_a sum_i W[i,a,k] * (x0T[i,n] * aT[a,n])
The inner operand U_a[i,n] = x0T[i,n]*aT[a,n] is built on the vector engine
as a tensor_tensor multiply against "Arep_a" = row a of aT broadcast across
128 partitions; Arep is produced on the tensor engine via a one-hot selector
matmul (out = sel_a.T @ aT = broadcast).
"""

import numpy as np
import ml_dtypes
from contextlib import ExitStack

import concourse.bass as bass
import concourse.mybir as mybir
import concourse.tile as tile
from concourse import bacc
from concourse.bass import ts
from concourse.bass_utils import run_bass_kernel_spmd

N_CORES = 8
N_FULL = 16384
NSH = N_FULL // N_CORES          # 2048 nodes per core
A = 16                           # attr dim
M0 = 128                         # MUL0 (scalar channels)
FREE = 512                       # node tile (free dim) per inner step
SCALE = 1.0 / np.sqrt(M0 * A)    # path normalization of both fctp einsums
BF16 = ml_dtypes.bfloat16

AF = mybir.ActivationFunctionType
F32 = mybir.dt.float32
DBF16 = mybir.dt.bfloat16


def build_nc(nsh: int = NSH, free: int = FREE, num_devices: int = N_CORES):
    nc = bacc.Bacc(
        "TRN2",
        target_bir_lowering=False,
        debug=False,
        enable_asserts=False,
        num_devices=num_devices,
    )

    x0t = nc.dram_tensor("x0t", [M0, nsh], DBF16, kind="ExternalInput").ap()
    atrep = nc.dram_tensor("atrep", [A * M0, nsh], DBF16, kind="ExternalInput").ap()
    w0 = nc.dram_tensor("w0", [M0, A * M0], DBF16, kind="ExternalInput").ap()
    w2 = nc.dram_tensor("w2", [M0, A * A], DBF16, kind="ExternalInput").ap()
    w3 = nc.dram_tensor("w3", [A, A], DBF16, kind="ExternalInput").ap()
    w4 = nc.dram_tensor("w4", [A, 1], DBF16, kind="ExternalInput").ap()
    b1 = nc.dram_tensor("b1", [M0, 1], F32, kind="ExternalInput").ap()
    b2 = nc.dram_tensor("b2", [A, 1], F32, kind="ExternalInput").ap()
    b3 = nc.dram_tensor("b3", [A, 1], F32, kind="ExternalInput").ap()
    b4 = nc.dram_tensor("b4", [1, 1], F32, kind="ExternalInput").ap()
    outt = nc.dram_tensor("outt", [1, nsh], F32, kind="ExternalOutput").ap()

    n_tiles = nsh // free

    with tile.TileContext(nc) as tc, ExitStack() as ctx:
        consts = ctx.enter_context(tc.tile_pool(name="consts", bufs=1))

        x0t_sb = consts.tile([M0, nsh], DBF16)
        nc.sync.dma_start(x0t_sb[:], x0t)
        w0_sb = consts.tile([M0, A * M0], DBF16)
        nc.sync.dma_start(w0_sb[:], w0)
        w2_sb = consts.tile([M0, A * A], DBF16)
        nc.sync.dma_start(w2_sb[:], w2)
        w3_sb = consts.tile([A, A], DBF16)
        nc.sync.dma_start(w3_sb[:], w3)
        w4_sb = consts.tile([A, 1], DBF16)
        nc.sync.dma_start(w4_sb[:], w4)
        b1_sb = consts.tile([M0, 1], F32)
        nc.sync.dma_start(b1_sb[:], b1)
        b2_sb = consts.tile([A, 1], F32)
        nc.sync.dma_start(b2_sb[:], b2)
        b3_sb = consts.tile([A, 1], F32)
        nc.sync.dma_start(b3_sb[:], b3)
        b4_sb = consts.tile([1, 1], F32)
        nc.sync.dma_start(b4_sb[:], b4)

        arep_pool = ctx.enter_context(tc.tile_pool(name="arep", bufs=2))
        u_pool = ctx.enter_context(tc.tile_pool(name="u", bufs=3))
        s_pool = ctx.enter_context(tc.tile_pool(name="s", bufs=2))
        o_pool = ctx.enter_context(tc.tile_pool(name="o", bufs=2))
        ps_s0 = ctx.enter_context(tc.tile_pool(name="ps_s0", bufs=2, space="PSUM"))
        ps_mid = ctx.enter_context(tc.tile_pool(name="ps_mid", bufs=2, space="PSUM"))
        ps_mlp = ctx.enter_context(tc.tile_pool(name="ps_mlp", bufs=1, space="PSUM"))

        for t in range(n_tiles):
            sl = ts(t, free)

            # Arep_a[p, n] = aT[a, n] for all p — host-replicated, plain DMA.
            arep = arep_pool.tile([M0, A * free], DBF16)
            for a in range(A):
                nc.sync.dma_start(
                    arep[:, ts(a, free)], atrep[ts(a, M0), sl]
                )

            # s0T accumulation over the 16 a-chunks of c=(a,i).
            s0_ps = ps_s0.tile([M0, free], F32)
            for a in range(A):
                u0 = u_pool.tile([M0, free], DBF16, tag="u0")
                nc.vector.tensor_mul(u0[:], x0t_sb[:, sl], arep[:, ts(a, free)])
                nc.tensor.matmul(
                    s0_ps[:], w0_sb[:, ts(a, M0)], u0[:],
                    start=(a == 0), stop=(a == A - 1),
                )

            # silu(s0 + b1) = (s0+b1) * sigmoid(s0+b1); CoreSim has no Silu LUT.
            s_pre = s_pool.tile([M0, free], DBF16, tag="s_pre")
            nc.scalar.activation(s_pre[:], s0_ps[:], AF.Identity, bias=b1_sb[:])
            s_sig = s_pool.tile([M0, free], DBF16, tag="s_sig")
            nc.scalar.activation(s_sig[:], s0_ps[:], AF.Sigmoid, bias=b1_sb[:])
            scal = s_pool.tile([M0, free], DBF16, tag="scal")
            nc.vector.tensor_mul(scal[:], s_pre[:], s_sig[:])

            # midT accumulation.
            mid_ps = ps_mid.tile([A, free], F32)
            for a in range(A):
                u3 = u_pool.tile([M0, free], DBF16, tag="u3")
                # split the multiplies across DVE and the idle GPSIMD
                eng = nc.vector if a % 2 == 0 else nc.gpsimd
                eng.tensor_mul(u3[:], scal[:], arep[:, ts(a, free)])
                nc.tensor.matmul(
                    mid_ps[:], w2_sb[:, ts(a, A)], u3[:],
                    start=(a == 0), stop=(a == A - 1),
                )

            midb = s_pool.tile([A, free], DBF16, tag="midb")
            nc.scalar.activation(midb[:], mid_ps[:], AF.Identity, bias=b2_sb[:])

            h_ps = ps_mlp.tile([A, free], F32, tag="h")
            nc.tensor.matmul(h_ps[:], w3_sb[:], midb[:], start=True, stop=True)
            h_pre = s_pool.tile([A, free], DBF16, tag="h_pre")
            nc.scalar.activation(h_pre[:], h_ps[:], AF.Identity, bias=b3_sb[:])
            h_sig = s_pool.tile([A, free], DBF16, tag="h_sig")
            nc.scalar.activation(h_sig[:], h_ps[:], AF.Sigmoid, bias=b3_sb[:])
            hb = s_pool.tile([A, free], DBF16, tag="hb")
            nc.vector.tensor_mul(hb[:], h_pre[:], h_sig[:])

            out_ps = ps_mlp.tile([1, free], F32, tag="out")
            nc.tensor.matmul(out_ps[:], w4_sb[:], hb[:], start=True, stop=True)
            ob = o_pool.tile([1, free], F32)
            nc.scalar.activation(ob[:], out_ps[:], AF.Identity, bias=b4_sb[:])
            nc.sync.dma_start(outt[:, sl], ob[:])

    nc.compile()
    return nc


def prep_host(inputs: dict, nsh: int = NSH, n_cores: int = N_CORES):
    """Host-side prep: slice/transpose/cast inputs, build per-core in_maps."""
    node_vec = np.asarray(inputs["node_vec"], dtype=np.float32)
    node_embedding = np.asarray(inputs["node_embedding"], dtype=np.float32)
    W1_l0 = np.asarray(inputs["W1_l0"], dtype=np.float32)
    b1 = np.asarray(inputs["b1"], dtype=np.float32)
    W2 = np.asarray(inputs["W2"], dtype=np.float32)
    b2 = np.asarray(inputs["b2"], dtype=np.float32)
    W3 = np.asarray(inputs["W3"], dtype=np.float32)
    b3 = np.asarray(inputs["b3"], dtype=np.float32)
    W4 = np.asarray(inputs["W4"], dtype=np.float32)
    b4 = np.asarray(inputs["b4"], dtype=np.float32)

    x0T = np.ascontiguousarray(node_vec[:, :M0].T).astype(BF16)      # [128, N]
    aT = np.ascontiguousarray(node_embedding.T).astype(BF16)         # [16, N]
    aTrep = np.ascontiguousarray(np.repeat(aT, M0, axis=0))          # [2048, N]

    w0h = (W1_l0[:, :, :M0] * SCALE).reshape(M0, A * M0).astype(BF16)
    w2h = (W2 * SCALE).reshape(M0, A * A).astype(BF16)
    w3h = (W3 / np.sqrt(A)).astype(BF16)
    w4h = (W4 / np.sqrt(A)).astype(BF16)

    shared = {
        "w0": w0h, "w2": w2h, "w3": w3h, "w4": w4h,
        "b1": np.ascontiguousarray(b1[:M0].reshape(M0, 1)),
        "b2": np.ascontiguousarray(b2.reshape(A, 1)),
        "b3": np.ascontiguousarray(b3.reshape(A, 1)),
        "b4": np.ascontiguousarray(b4.reshape(1, 1)),
    }
    in_maps = []
    for c in range(n_cores):
        sl = slice(c * nsh, (c + 1) * nsh)
        in_maps.append({
            "x0t": np.ascontiguousarray(x0T[:, sl]),
            "atrep": np.ascontiguousarray(aTrep[:, sl]),
            **shared,
        })
    return in_maps


_NC_CACHE = {}


def _get_nc():
    if "nc" not in _NC_CACHE:
        _NC_CACHE["nc"] = build_nc()
    return _NC_CACHE["nc"]


def kernel_with_results(trace: bool = False, **inputs):
    nc = _get_nc()
    in_maps = prep_host(inputs)
    res = run_bass_kernel_spmd(
        nc, in_maps, core_ids=list(range(N_CORES)), trace=trace,
    )
    out = np.empty((N_FULL, 1), dtype=np.float32)
    for c in range(N_CORES):
        out[c * NSH:(c + 1) * NSH, 0] = res.results[c]["outt"][0]
    return out, res


def kernel(**inputs) -> np.ndarray:
    out, _ = kernel_with_results(trace=False, **inputs)
    return out



# revision 10
# speedup vs baseline: 1.6858x; 1.6858x over previous
"""Trainium2 Bass kernel for nn_InvariantHeadviaTP.

Reference computation (after dead-code elimination -- y1/y2/gates are never
used by the output):
    x0   = node_vec[:, :128]                  # [N, 128]
    a    = node_embedding                     # [N, 16]
    s0   = einsum('ni,na,iak->nk', x0, a, W1_l0[:, :, :128]) / sqrt(2048) + b1[:128]
    scal = silu(s0)                           # [N, 128]
    mid  = einsum('ni,na,iak->nk', scal, a, W2) / sqrt(2048) + b2   # [N, 16]
    h    = silu(mid @ W3 / 4 + b3)            # [N, 16]
    out  = h @ W4 / 4 + b4                    # [N, 1]

Strategy (data-parallel, 2048 nodes/core, transposed layout: features on
partitions, nodes on the free dim):

s0 path -- mixed (i,a) chunking. The contraction index c=(i,a) [128*16=2048]
is split into 16 chunks of 128 = (32 i's) x (4 a's); chunk (bi,bj) has
partition p = 4*i_loc + a_loc. The elementwise operand
U_c[p,n] = x0[32bi+p//4, n] * a[4bj+p%4, n] is built on DVE from
  x0rep_bi[p,n] = x0t[32bi+p//4, n]   (x0 rows repeated 4x  -- stride-0 DMA)
  pats_bj [p,n] = aT [4bj+p%4,  n]    (a rows tiled 32x     -- stride-0 DMA)
so the replicated-broadcast traffic is 4 MB/core instead of the naive 8 MB
(a replicated to all 128 partitions). s0 accumulates over the 16 chunks in
PSUM via 16 matmuls per 512-node tile.

silu is a single hardware Activation op (Silu is in the same HW table as
Identity). CoreSim has no Silu LUT, so sim_silu=True builds an
Identity+Sigmoid+mul equivalent for simulation.

mid path -- only 16 outputs, so instead of another 16-matmul bilinear:
  g[(a,k'),n] = sum_i scal[i,n] W2[i,a,k']        (2 matmuls, stat [128,128])
  v = g * patm,  patm[(a,k'),n] = aT[a,n]         (2 DVE muls)
  h_pre[k2,n] = sum_(a,k') v * W3[k',k2]          (2 matmuls, W3 fused into
                                                   the a-sum selector)
with b2 folded into b3' = b3 + b2 @ W3s host-side, and b4 folded into an
augmented ones-row of the final W4 matmul.

The core runs in two 1024-node halves so half 0's mid path overlaps half 1's
s0 accumulation.
"""

import os
import numpy as np
import ml_dtypes
from contextlib import ExitStack

import concourse.bass as bass
import concourse.mybir as mybir
import concourse.tile as tile
from concourse import bacc
from concourse.bass import ts
from concourse.bass_utils import run_bass_kernel_spmd

N_CORES = 8
N_FULL = 16384
NSH = N_FULL // N_CORES          # 2048 nodes per core
A = 16                           # attr dim
M0 = 128                         # MUL0 (scalar channels)
FREE = 512                       # node tile (free dim) per PSUM tile
HALF = 1024                      # nodes per half-phase
SCALE = 1.0 / np.sqrt(M0 * A)    # path normalization of both fctp einsums
BF16 = ml_dtypes.bfloat16

AF = mybir.ActivationFunctionType
F32 = mybir.dt.float32
DBF16 = mybir.dt.bfloat16


def build_nc(nsh: int = NSH, num_devices: int = N_CORES, sim_silu: bool = False):
    assert nsh % HALF == 0
    n_halves = nsh // HALF

    nc = bacc.Bacc(
        "TRN2",
        target_bir_lowering=False,
        debug=False,
        enable_asserts=False,
        num_devices=num_devices,
    )

    x0t = nc.dram_tensor("x0t", [M0, nsh], DBF16, kind="ExternalInput").ap()
    at = nc.dram_tensor("at", [A, nsh], DBF16, kind="ExternalInput").ap()
    w0f = nc.dram_tensor("w0f", [M0, 16 * M0], DBF16, kind="ExternalInput").ap()
    w2g = nc.dram_tensor("w2g", [M0, 2 * M0], DBF16, kind="ExternalInput").ap()
    s3 = nc.dram_tensor("s3", [M0, A], DBF16, kind="ExternalInput").ap()
    w4a = nc.dram_tensor("w4a", [A, 1], DBF16, kind="ExternalInput").ap()
    b1 = nc.dram_tensor("b1", [M0, 1], F32, kind="ExternalInput").ap()
    b3p = nc.dram_tensor("b3p", [A, 1], F32, kind="ExternalInput").ap()
    b4 = nc.dram_tensor("b4", [1, 1], F32, kind="ExternalInput").ap()
    outt = nc.dram_tensor("outt", [1, nsh], F32, kind="ExternalOutput").ap()

    with tile.TileContext(nc) as tc, ExitStack() as ctx:
        consts = ctx.enter_context(tc.tile_pool(name="consts", bufs=1))

        # -- SBUF residents -------------------------------------------------
        x0rep = []
        for bi in range(4):
            x0rep.append(consts.tile([M0, nsh], DBF16, name=f"x0rep{bi}"))
        pats = consts.tile([M0, 4 * nsh], DBF16)      # bj-major a-patterns
        patm = consts.tile([M0, 2 * nsh], DBF16)      # mid-path a-patterns
        w0_sb = consts.tile([M0, 16 * M0], DBF16)
        w2g_sb = consts.tile([M0, 2 * M0], DBF16)
        s3_sb = consts.tile([M0, A], DBF16)
        w4a_sb = consts.tile([A, 1], DBF16)
        b1_sb = consts.tile([M0, 1], F32)
        b3p_sb = consts.tile([A, 1], F32)
        b4_sb = consts.tile([1, 1], F32)
        scal_sb = consts.tile([M0, nsh], DBF16)
        hb_all = consts.tile([A, nsh], DBF16)
        ob = consts.tile([1, nsh], F32)

        # -- prefetch DMAs (interleaved so half-0 unblocks early) -----------
        # x0rep_bi[p, n] = x0t[32*bi + p//4, n]
        nc.sync.dma_start(
            x0rep[0][:],
            x0t[0:32, :].unsqueeze(1).broadcast_to([32, 4, nsh]),
        )
        # pats block bj: [p, n] = aT[4*bj + p%4, n]
        for bj in range(4):
            nc.scalar.dma_start(
                pats[:, ts(bj, nsh)],
                at[4 * bj:4 * bj + 4, :].unsqueeze(0).broadcast_to([32, 4, nsh]),
            )
        for bi in range(1, 4):
            nc.sync.dma_start(
                x0rep[bi][:],
                x0t[32 * bi:32 * bi + 32, :].unsqueeze(1).broadcast_to([32, 4, nsh]),
            )
        nc.sync.dma_start(w0_sb[:], w0f)
        # patm block b: [p, n] = aT[8*b + p//16, n]
        for b in range(2):
            nc.scalar.dma_start(
                patm[:, ts(b, nsh)],
                at[8 * b:8 * b + 8, :].unsqueeze(1).broadcast_to([8, 16, nsh]),
            )
        nc.scalar.dma_start(w2g_sb[:], w2g)
        nc.scalar.dma_start(s3_sb[:], s3)
        nc.scalar.dma_start(w4a_sb[:], w4a)
        nc.scalar.dma_start(b1_sb[:], b1)
        nc.scalar.dma_start(b3p_sb[:], b3p)
        nc.scalar.dma_start(b4_sb[:], b4)

        pats3 = pats[:].rearrange("p (b n) -> p b n", b=4)   # [128, 4, nsh]

        u_pool = ctx.enter_context(tc.tile_pool(name="u", bufs=3))
        v_pool = ctx.enter_context(tc.tile_pool(name="v", bufs=2))
        sim_pool = (
            ctx.enter_context(tc.tile_pool(name="simtmp", bufs=2))
            if sim_silu else None
        )
        ps_s0 = ctx.enter_context(tc.tile_pool(name="ps_s0", bufs=1, space="PSUM"))
        ps_g = ctx.enter_context(tc.tile_pool(name="ps_g", bufs=1, space="PSUM"))
        ps_h = ctx.enter_context(tc.tile_pool(name="ps_h", bufs=1, space="PSUM"))
        ps_o = ctx.enter_context(tc.tile_pool(name="ps_o", bufs=1, space="PSUM"))

        def silu(out_ap, in_ap, bias_ap, tmp_shape):
            if not sim_silu:
                nc.scalar.activation(out_ap, in_ap, AF.Silu, bias=bias_ap)
            else:
                pre = sim_pool.tile([M0, FREE], DBF16, tag="pre", name="pre")
                sig = sim_pool.tile([M0, FREE], DBF16, tag="sig", name="sig")
                p = tmp_shape[0]
                nc.scalar.activation(pre[0:p, :], in_ap, AF.Identity, bias=bias_ap)
                nc.scalar.activation(sig[0:p, :], in_ap, AF.Sigmoid, bias=bias_ap)
                nc.vector.tensor_mul(out_ap, pre[0:p, :], sig[0:p, :])

        for h in range(n_halves):
            nsl = ts(h, HALF)

            # ---- phase A: build U, accumulate s0 for tiles 2h, 2h+1 ----
            s0_ps = [
                ps_s0.tile([M0, FREE], F32, tag=f"s0_{h}_{t2}", name=f"s0_{h}_{t2}")
                for t2 in range(2)
            ]
            u_tiles = []
            for bi in range(4):
                u = u_pool.tile([M0, 4, HALF], DBF16, tag="u", name=f"u{h}_{bi}")
                in0 = x0rep[bi][:, nsl].unsqueeze(1)
                in1 = pats3[:, :, nsl]
                if bi < 3:
                    nc.vector.tensor_mul(
                        u[:], in0.broadcast_to([M0, 4, HALF]), in1
                    )
                else:
                    # split the last block between DVE and GPSIMD
                    nc.vector.tensor_mul(
                        u[:, 0:2, :],
                        in0.broadcast_to([M0, 2, HALF]),
                        in1[:, 0:2, :],
                    )
                    nc.gpsimd.tensor_mul(
                        u[:, 2:4, :],
                        in0.broadcast_to([M0, 2, HALF]),
                        in1[:, 2:4, :],
                    )
                u_tiles.append(u)

            for c in range(16):
                bi, bj = c >> 2, c & 3
                for t2 in range(2):
                    nc.tensor.matmul(
                        s0_ps[t2][:],
                        w0_sb[:, ts(c, M0)],
                        u_tiles[bi][:, bj, ts(t2, FREE)],
                        start=(c == 0),
                        stop=(c == 15),
                    )

            # ---- phase B: mid path for tiles 2h, 2h+1 ----
            for t2 in range(2):
                t = 2 * h + t2
                sl = ts(t, FREE)

                silu(scal_sb[:, sl], s0_ps[t2][:], b1_sb[:], [M0, FREE])

                g_ps = []
                for gi in range(2):
                    g = ps_g.tile([M0, FREE], F32, tag=f"g{gi}")
                    nc.tensor.matmul(
                        g[:], w2g_sb[:, ts(gi, M0)], scal_sb[:, sl],
                        start=True, stop=True,
                    )
                    g_ps.append(g)

                h_ps = ps_h.tile([A, FREE], F32, tag="h")
                for gi in range(2):
                    v = v_pool.tile([M0, FREE], DBF16, tag=f"v{gi}")
                    nc.vector.tensor_mul(
                        v[:], g_ps[gi][:],
                        patm[:, gi * nsh + t * FREE:gi * nsh + (t + 1) * FREE],
                    )
                    nc.tensor.matmul(
                        h_ps[:], s3_sb[:], v[:],
                        start=(gi == 0), stop=(gi == 1),
                    )

                silu(hb_all[:, sl], h_ps[:], b3p_sb[:], [A, FREE])

                o_ps = ps_o.tile([1, FREE], F32, tag="o")
                nc.tensor.matmul(
                    o_ps[:], w4a_sb[:], hb_all[:, sl], start=True, stop=True,
                )
                nc.scalar.activation(ob[:, sl], o_ps[:], AF.Identity, bias=b4_sb[:])
                nc.sync.dma_start(outt[:, sl], ob[:, sl])

    nc.compile()
    return nc


def prep_host(inputs: dict, nsh: int = NSH, n_cores: int = N_CORES):
    """Host-side prep: slice/transpose/cast inputs, build per-core in_maps."""
    node_vec = np.asarray(inputs["node_vec"], dtype=np.float32)
    node_embedding = np.asarray(inputs["node_embedding"], dtype=np.float32)
    W1_l0 = np.asarray(inputs["W1_l0"], dtype=np.float32)
    b1 = np.asarray(inputs["b1"], dtype=np.float32)
    W2 = np.asarray(inputs["W2"], dtype=np.float32)
    b2 = np.asarray(inputs["b2"], dtype=np.float32)
    W3 = np.asarray(inputs["W3"], dtype=np.float32)
    b3 = np.asarray(inputs["b3"], dtype=np.float32)
    W4 = np.asarray(inputs["W4"], dtype=np.float32)
    b4 = np.asarray(inputs["b4"], dtype=np.float32)

    x0T = np.ascontiguousarray(node_vec[:, :M0].T).astype(BF16)      # [128, N]
    aT = np.ascontiguousarray(node_embedding.T).astype(BF16)         # [16, N]

    W0s = W1_l0[:, :, :M0] * SCALE                                   # [128,16,128]
    # chunk (bi,bj): stationary rows p=(i_loc*4 + a_loc), cols (c*128 + k)
    w0r = W0s.reshape(4, 32, 4, 4, M0)            # [bi, i_loc, bj, a_loc, k]
    w0f = np.ascontiguousarray(
        w0r.transpose(1, 3, 0, 2, 4).reshape(M0, 16 * M0)
    ).astype(BF16)

    W3s = W3 / np.sqrt(A)                                            # [16, 16]
    w2g = np.ascontiguousarray((W2 * SCALE).reshape(M0, A * A)).astype(BF16)
    s3h = np.ascontiguousarray(np.tile(W3s, (8, 1))).astype(BF16)    # [128, 16]
    w4a = (W4 / np.sqrt(A)).astype(BF16)                             # [16, 1]
    b3ph = (b3 + b2 @ W3s).reshape(A, 1).astype(np.float32)

    shared = {
        "w0f": w0f, "w2g": w2g, "s3": s3h, "w4a": w4a,
        "b1": np.ascontiguousarray(b1[:M0].reshape(M0, 1)),
        "b3p": b3ph,
        "b4": np.ascontiguousarray(b4.reshape(1, 1)),
    }
    in_maps = []
    for c in range(n_cores):
        sl = slice(c * nsh, (c + 1) * nsh)
        in_maps.append({
            "x0t": np.ascontiguousarray(x0T[:, sl]),
            "at": np.ascontiguousarray(aT[:, sl]),
            **shared,
        })
    return in_maps


_NC_CACHE = {}


def _get_nc():
    if "nc" not in _NC_CACHE:
        _NC_CACHE["nc"] = build_nc()
    return _NC_CACHE["nc"]


def kernel_with_results(trace: bool = False, **inputs):
    nc = _get_nc()
    in_maps = prep_host(inputs)
    res = run_bass_kernel_spmd(
        nc, in_maps, core_ids=list(range(N_CORES)), trace=trace,
    )
    out = np.empty((N_FULL, 1), dtype=np.float32)
    for c in range(N_CORES):
        out[c * NSH:(c + 1) * NSH, 0] = res.results[c]["outt"][0]
    return out, res


def kernel(**inputs) -> np.ndarray:
    out, _ = kernel_with_results(trace=False, **inputs)
    return out
